# revision 13
# baseline (speedup 1.0000x reference)
"""Grouped-correlation cost volume (CostVolume) Bass kernel for Trainium2.

Problem: x, y: (4, 512, 128, 256) f32; GROUP=4, MAXDISP=48, D=49.
out[b, g, k, h, w] = sum_cg x[b, 128g+cg, h, w] * y[b, 128g+cg, h, w-k]
(zero where w < k), out shape (4, 4, 49, 128, 256).

Strategy: shard the 16 (b, g) units over 8 cores (2 each; the channel sum is
within-group, so no cross-core reduce). Per (unit, h) row the correlation is
a banded Gram matrix between x columns and y columns with contraction over
cg = 128 = the TensorE partition dim. Each 128-wide w-block is split into
four M=32 column groups (tile_position col-tiling) whose y-windows are
shifted by the group base:

  P[32m+i', 80t+j'] = sum_cg x[cg, 128t+32m+i'] * y[cg, 128t+32m-48+j']

so the useful entries are j' = i' + 48 - k with i' in [0,32), j' in [0,80) —
a 32x80 parallelogram per group (1.63x amplification instead of 3.6x for
M=128). The per-(unit,h) (128, 160) PSUM rows are cast to fp16 in SBUF and
stored to DRAM as-is; the band extraction (a pure gather) happens on the
host during the unshard step.

Precision: the whole pipeline runs in fp16 (inputs are cast on the host,
matmul accumulates in fp32 PSUM, the rect is stored as fp16). This halves
every DMA stream and quadruples TensorE throughput vs fp32; the resulting
relative error is ~4e-4, far inside the 2e-2 gate.

DMA layout: x and y rows are interleaved per channel in ONE input tensor,
so each h-chunk is a single load DMA with 16KB descriptors (per-load fixed
costs — ~1us completion descriptors pinned to DMA engine 0 — halve). y is
NOT zero-padded: windows that would read y[h, w<0] read the previous row's
tail (or x's tail / SBUF slack) instead of zeros — those products only land
in band entries with w < k, whose reference value is exactly 0, so the host
unshard step zeroes them unconditionally.

Scheduling: loads issue from SP, stores from Pool — separate in-order
sequencers so neither stream stalls the other. PSUM->SBUF cast-copies
alternate DVE / Activation. The last two chunks stream out in 8-row store
pieces as their copies land, shortening the post-final-load tail.

The module is built through bacc (not raw bass) so excess semaphore waits
get split onto EventSemaphore instructions — TRN2 allows at most one sync
wait per regular instruction.
"""

import os

import numpy as np

import concourse.bass as bass
import concourse.mybir as mybir
import concourse.tile as tile
from concourse import bacc

MAXDISP = 48
D = MAXDISP + 1          # 49 disparities
CG = 128                 # channels per group = contraction dim
GROUP = 4
B = 4
H = 128
W = 256
NB = W // 128            # 2 w-blocks of 128
NM = 4                   # M=32 col groups per w-block
MW = 32                  # group width
NWIN = MAXDISP + MW      # 80: y window per group
RECT_W = NB * NWIN       # 160 stored columns per (unit, h)
N_CORES = 8
N_UNITS = 2              # (b,g) units per core
H_CHUNK = 32
N_CHUNKS = H // H_CHUNK
H_PAIR = 2               # h rows per PSUM tile / copy

_last_results = None     # BassKernelResults of the most recent run (for test.py)


def build_nc(n_units=N_UNITS, n_h=H, h_chunk=H_CHUNK):
    """Build the per-core Bass module (fp16 IO, fp32 PSUM accumulate)."""
    assert n_h % h_chunk == 0
    n_chunks = n_h // h_chunk
    f16 = mybir.dt.float16
    f32 = mybir.dt.float32
    hcw = h_chunk * W
    flat_len = 2 * hcw + 48   # x region | y slack(48) | y region

    nc = bacc.Bacc()
    xy = nc.dram_tensor("xy", [n_units, CG, 2, n_h * W], f16, kind="ExternalInput")
    out = nc.dram_tensor(
        "out", [n_units, n_chunks, 128, h_chunk * RECT_W], f16,
        kind="ExternalOutput",
    )

    with tile.TileContext(nc) as tc:
        with (
            tc.tile_pool(name="io", bufs=4) as io_pool,
            tc.tile_pool(name="work", bufs=4) as work_pool,
            tc.tile_pool(name="psum_mm", bufs=6, space="PSUM") as psum_mm,
        ):
            for u in range(n_units):
                for hc in range(n_chunks):
                    h0 = hc * h_chunk
                    tail = u == n_units - 1 and hc >= n_chunks - 2
                    xy_tile = io_pool.tile(
                        [128, flat_len], f16, name="xy_tile", tag="xy"
                    )
                    # one DMA per chunk: x rows at [0, hcw), y rows at
                    # [hcw+48, 2*hcw+48) — the 48-col slack between them only
                    # feeds host-zeroed w<k outputs
                    xy_dst = bass.AP(
                        tensor=xy_tile.tensor,
                        offset=xy_tile.offset,
                        ap=[[flat_len, 128], [hcw + 48, 2], [1, hcw]],
                    )
                    nc.sync.dma_start(
                        out=xy_dst, in_=xy[u, :, :, h0 * W : h0 * W + hcw]
                    )

                    # per-chunk staging tile so the store is one big DMA
                    s_big = work_pool.tile(
                        [128, h_chunk * RECT_W], f16, name="s_big", tag="S"
                    )
                    for hp in range(h_chunk // H_PAIR):
                        p_mm = psum_mm.tile(
                            [128, H_PAIR * RECT_W], f32, name="p_mm", tag="P"
                        )
                        for hh in range(H_PAIR):
                            h = hp * H_PAIR + hh
                            for t in range(NB):
                                for m in range(NM):
                                    base = 128 * t + MW * m
                                    lhsT = xy_tile[:, h * W + base : h * W + base + MW]
                                    # rhs covers y cols [base-48, base+32) of
                                    # row h: tile col hcw+48 + h*W + base - 48
                                    rhs = xy_tile[
                                        :, hcw + h * W + base : hcw + h * W + base + NWIN
                                    ]
                                    nc.tensor.matmul(
                                        p_mm[
                                            MW * m : MW * (m + 1),
                                            hh * RECT_W + NWIN * t :
                                            hh * RECT_W + NWIN * (t + 1),
                                        ],
                                        lhsT,
                                        rhs,
                                        start=True,
                                        stop=True,
                                        tile_position=(0, MW * m),
                                    )
                        dst = s_big[
                            :, hp * H_PAIR * RECT_W : (hp + 1) * H_PAIR * RECT_W
                        ]
                        # alternate cast-copy between DVE and Activation
                        if hp % 2 == 0:
                            nc.vector.tensor_copy(dst, p_mm)
                        else:
                            nc.scalar.copy(dst, p_mm)
                        # tail taper: the final chunks stream out in 8-row
                        # pieces as copies land instead of one store at the
                        # end, so little work remains after the last load
                        if tail and hp % 4 == 3:
                            q0 = (hp - 3) * H_PAIR * RECT_W
                            q1 = (hp + 1) * H_PAIR * RECT_W
                            nc.gpsimd.dma_start(
                                out=out[u, hc][:, q0:q1], in_=s_big[:, q0:q1]
                            )
                    # stores on their own engine: an in-order sequencer that
                    # also issued loads would stall them behind store waits
                    if not tail:
                        nc.gpsimd.dma_start(out=out[u, hc], in_=s_big)

    nc.finalize()
    return nc


def _shard_inputs(x, y):
    """x, y: (4, 512, 128, 256) f16 -> per-core dicts with one interleaved
    xy tensor of shape (2, 128, 2, H*W)."""
    xu = x.reshape(B * GROUP, CG, H * W)
    yu = y.reshape(B * GROUP, CG, H * W)
    xy = np.stack([xu, yu], axis=2)  # (16, CG, 2, H*W)
    return [
        {"xy": np.ascontiguousarray(xy[2 * c : 2 * c + 2])} for c in range(N_CORES)
    ]


def _extract_band(rect, n_h=H):
    """rect: (n, n_chunks, 128, h_chunk*160) rects -> (n, D, n_h, W) f32.

    rect[n, hc, 32m+i, (h'*160)+80t+j] = out[n, 48-(j-i), hc*h_chunk+h',
    128t+32m+i] for j-i in [0, 48]; entries with w < k are garbage (they
    read across y row boundaries) and are overwritten with the reference's
    exact zeros.
    """
    n, n_chunks, _, _ = rect.shape
    h_chunk = n_h // n_chunks
    r = rect.reshape(n, n_chunks, NM, MW, h_chunk, NB, NWIN)  # [n,hc,m,i,h',t,j]
    idx = np.arange(MW)[:, None] + np.arange(D)[None, :]      # j = i + c
    g = np.take_along_axis(
        r, idx[None, None, None, :, None, None, :], axis=-1
    )  # [n, hc, m, i, h', t, c]
    g = g.transpose(0, 6, 1, 4, 5, 2, 3)                      # [n,c,hc,h',t,m,i]
    g = g.reshape(n, D, n_h, W)[:, ::-1]                      # c -> k = 48 - c
    g = np.ascontiguousarray(g, dtype=np.float32)
    for k in range(1, D):                                     # out[..,k,:,w<k] = 0
        g[:, k, :, :k] = 0.0
    return g


def kernel(x, y):
    global _last_results
    from concourse.bass_utils import run_bass_kernel_spmd

    x = np.asarray(x, dtype=np.float32).astype(np.float16)
    y = np.asarray(y, dtype=np.float32).astype(np.float16)

    nc = build_nc()
    in_maps = _shard_inputs(x, y)
    trace = bool(int(os.environ.get("COSTVOL_TRACE", "0")))
    results = run_bass_kernel_spmd(
        nc,
        in_maps,
        core_ids=list(range(N_CORES)),
        trace=trace,
    )
    _last_results = results

    rects = np.concatenate([r["out"] for r in results.results], axis=0)
    full = _extract_band(rects)  # (16, D, H, W) f32
    return full.reshape(B, GROUP, D, H, W)
